# revision 10
# baseline (speedup 1.0000x reference)
"""Trainium2 Bass kernel for nn_EqPBC (triplet-feature PBC equalizer), v2.

Data-parallel over 8 NeuronCores, 8192 samples/core, 16 chunks of 512.
Per chunk (batch on free dim, features on partitions):
  1. DMA [128,82] f32 blocks, PE-transpose -> etT [82,512] bf16 per comp
  2. One-hot gather matmuls (PE) -> En/Em/Emn rows (p,h); PSUM->SBUF bf16
  3. DVE products + combines: q1=S_r, q2=S_i per (sname in {S1,S2}, j-block)
  4. p-folds: a-blocks on DVE (j0+j2), packed b-tails via PE fold matmul
  5. T-matmuls (PE): T[i,c] = sum_h W'[i,h]*(S1[h]->c=m_h) + (S2[h]->c=n_h)
     with host-built weight matrices; T_r/T_i psum [82,512]
  6. final: Eout = sum_c E[c,i]*T[i,c] -> 4 DVE products + ones-matmuls
  7. out = E_L + Eout * 10^(ti0/10)/2 in f32 (exact E_L), DMA out

OOB Emn indices replicate JAX gather semantics (wrap negatives then clamp).
"""
import numpy as np
import ml_dtypes
from contextlib import ExitStack

# ----- static problem constants (hardcoded; kernel.py must be self-contained) -----
M = 41
L = M // 2
NMODES = 2
B = 65536
NCORES = 8
BC = B // NCORES          # 8192 samples per core
NB = 512                  # samples per chunk
NCHUNK = BC // NB         # 16
THRESH = 1.0 * M // 2
_idx = [(m, n) for m in range(-L, L + 1) for n in range(m, L + 1) if abs(m * n) <= THRESH]
M_ARR = np.array([m for m, n in _idx], dtype=np.int32)
N_ARR = np.array([n for m, n in _idx], dtype=np.int32)
DIAG = np.array([m == n for m, n in _idx])
HDIM = len(_idx)          # 177
HA = 128                  # h-split: a block [0,128), b block [128,177)
HB = HDIM - HA            # 49
BP = 113                  # packed b-block rows: p0 tail at 0:49, p1 tail at 64:113
GCOLS = 128 + BP + 128    # 369
TROWS = 64 + HB           # 113: folded-tail psum rows (q1 at 0:49, q2 at 64:113)

bf16 = ml_dtypes.bfloat16


def _gather_cols(idx_arr):
    """Source row in etT[82,:] (row = 2*(L+c) + p) for gathered rows (p,h)."""
    src = np.empty((2, HDIM), dtype=np.int64)
    for p in range(2):
        src[p] = 2 * (L + idx_arr) + p
    return src


def _build_consts():
    mn = L + M_ARR + N_ARR
    mn = np.clip(np.where(mn < 0, mn + M, mn), 0, M - 1) - L  # jax wrap+clamp
    srcs = {"n": _gather_cols(N_ARR), "m": _gather_cols(M_ARR), "mn": _gather_cols(mn)}
    gmats = {}
    for k, src in srcs.items():
        G = np.zeros((82, GCOLS), dtype=np.float32)
        for p in range(2):
            for h in range(HA):                      # a-blocks
                G[src[p, h], (0 if p == 0 else 128 + BP) + h] = 1.0
            for r in range(HB):                      # packed b-block
                G[src[p, HA + r], 128 + (0 if p == 0 else 64) + r] = 1.0
        gmats[k] = G.astype(bf16)
    return gmats


def _build_ffold2():
    """[113, 98] bf16 used as two [113,49] folds: out[r] = in[r] + in[64+r]."""
    F = np.zeros((BP, HB), dtype=np.float32)
    for r in range(HB):
        F[r, r] = 1.0
        F[64 + r, r] = 1.0
    return F.astype(bf16)


def _build_tmats(Wr, Wi):
    """Weight matrices for the T-stage matmuls.

    T[i,c] = sum_{h: m_h=c} W'[i,h] S1[h] + sum_{h: n_h=c} W'[i,h] S2[h]
    (W' = W * 0.5 on diag).  T row layout = 2*cfreq + i, cfreq = L + c.
    Returns dict of lhsT mats:
      ('a', sname, qk, tgt): [128, 82]  for p-folded a-block q tiles
      ('t', sname, tgt):     [98, 82]   for folded-tail tiles (q1 0:49, q2 49:98)
    tgt in {'r','i'}; q1 rows are S_r, q2 rows are S_i:
      T_r <- +W'_r*q1 - W'_i*q2 ; T_i <- +W'_i*q1 + W'_r*q2
    """
    scale = np.where(DIAG, 0.5, 1.0).astype(np.float32)
    wr = Wr * scale[None, :]  # [2, HDIM]
    wi = Wi * scale[None, :]
    cidx = {"S1": L + M_ARR, "S2": L + N_ARR}  # output freq index per h
    mats = {}
    for sname in ("S1", "S2"):
        c = cidx[sname]
        for tgt in ("r", "i"):
            # a-block: rows h in [0,128)
            m1 = np.zeros((HA, 82), dtype=np.float32)   # for q1 (S_r)
            m2 = np.zeros((HA, 82), dtype=np.float32)   # for q2 (S_i)
            for h in range(HA):
                for i in range(2):
                    col = 2 * c[h] + i
                    if tgt == "r":
                        m1[h, col] += wr[i, h]
                        m2[h, col] += -wi[i, h]
                    else:
                        m1[h, col] += wi[i, h]
                        m2[h, col] += wr[i, h]
            mats[("a", sname, "q1", tgt)] = m1.astype(bf16)
            mats[("a", sname, "q2", tgt)] = m2.astype(bf16)
            # tail: rows 0:49 = q1 (h=128+r), rows 64:113 = q2
            mt = np.zeros((TROWS, 82), dtype=np.float32)
            for r in range(HB):
                h = HA + r
                for i in range(2):
                    col = 2 * c[h] + i
                    if tgt == "r":
                        mt[r, col] += wr[i, h]
                        mt[64 + r, col] += -wi[i, h]
                    else:
                        mt[r, col] += wi[i, h]
                        mt[64 + r, col] += wr[i, h]
            mats[("t", sname, tgt)] = mt.astype(bf16)
    return mats


def _build_fmats():
    """Final reduce mats [82,4] bf16. out rows: 0=m0r,1=m0i,2=m1r,3=m1i.
    out_r[i] = sum_c (PR1 - PR2)[2c+i]; out_i[i] = sum_c (PI1 + PI2)[2c+i]."""
    fr1 = np.zeros((82, 4), dtype=np.float32)
    fr2 = np.zeros((82, 4), dtype=np.float32)
    fi = np.zeros((82, 4), dtype=np.float32)
    for c in range(M):
        for i in range(2):
            fr1[2 * c + i, 2 * i] = 1.0
            fr2[2 * c + i, 2 * i] = -1.0
            fi[2 * c + i, 2 * i + 1] = 1.0
    return fr1.astype(bf16), fr2.astype(bf16), fi.astype(bf16)


# engine assignment for gather-tile PSUM->SBUF copies: (kind, comp, j) -> "A"|"P"
def _copy_assignment():
    # Pool cannot read PSUM; split copies between Act ("A") and DVE ("D")
    asg = {}
    dve_set = {("n", "r", 1), ("n", "i", 1), ("m", "r", 1), ("m", "i", 1)}
    for kind in ("n", "m", "mn"):
        for comp in ("r", "i"):
            for j in range(3):
                asg[(kind, comp, j)] = "D" if (kind, comp, j) in dve_set else "A"
    return asg


# products that run on Pool instead of DVE: set of (sname, j, k) with k in 0..3
POOL_PRODUCTS = {("S2", j, k) for j in range(3) for k in range(4)}


def _build_kernel():
    import concourse.bass as bass
    import concourse.bacc as bacc
    import concourse.tile as tile
    import concourse.mybir as mybir

    dt = mybir.dt
    nc = bacc.Bacc("TRN2", target_bir_lowering=False, debug=False, num_devices=NCORES)
    xr = nc.declare_dram_parameter("xr", [BC, 82], dt.float32, isOutput=False)
    xi = nc.declare_dram_parameter("xi", [BC, 82], dt.float32, isOutput=False)
    ti = nc.declare_dram_parameter("ti", [BC, 4], dt.float32, isOutput=False)
    gn_d = nc.declare_dram_parameter("gn", [82, GCOLS], dt.bfloat16, isOutput=False)
    gm_d = nc.declare_dram_parameter("gm", [82, GCOLS], dt.bfloat16, isOutput=False)
    gmn_d = nc.declare_dram_parameter("gmn", [82, GCOLS], dt.bfloat16, isOutput=False)
    # T-stage weight mats packed: a-mats [128, 8*82], tail-mats [98, 4*82]
    wa_d = nc.declare_dram_parameter("wa", [HA, 8 * 82], dt.bfloat16, isOutput=False)
    wt_d = nc.declare_dram_parameter("wt", [TROWS, 4 * 82], dt.bfloat16, isOutput=False)
    fm_d = nc.declare_dram_parameter("fm", [82, 12], dt.bfloat16, isOutput=False)
    ffold_d = nc.declare_dram_parameter("ffold", [BP, HB], dt.bfloat16, isOutput=False)
    id128_d = nc.declare_dram_parameter("id128", [128, 128], dt.float32, isOutput=False)
    id4_d = nc.declare_dram_parameter("id4", [4, 4], dt.float32, isOutput=False)
    out_d = nc.declare_dram_parameter("out", [BC, 4], dt.float32, isOutput=True)

    LN10_10 = float(np.log(10.0) / 10.0)
    LNHALF = float(np.log(0.5))

    # a-mat column layout: idx = (sname,qk,tgt) -> slot
    a_slot = {}
    t_slot = {}
    s = 0
    for sname in ("S1", "S2"):
        for qk in ("q1", "q2"):
            for tgt in ("r", "i"):
                a_slot[(sname, qk, tgt)] = s
                s += 1
    s = 0
    for sname in ("S1", "S2"):
        for tgt in ("r", "i"):
            t_slot[(sname, tgt)] = s
            s += 1

    copy_asg = _copy_assignment()
    jrows = [HA, BP, HA]

    with tile.TileContext(nc) as tc, ExitStack() as ctx:
        cpool = ctx.enter_context(tc.tile_pool(name="consts", bufs=1))
        nat_pool = ctx.enter_context(tc.tile_pool(name="nat", bufs=2))
        et_pool = ctx.enter_context(tc.tile_pool(name="et", bufs=2))
        g_pool = ctx.enter_context(tc.tile_pool(name="gath", bufs=2))
        pr_pool = ctx.enter_context(tc.tile_pool(name="prod", bufs=2))
        q_pool = ctx.enter_context(tc.tile_pool(name="qt", bufs=2))
        t_pool = ctx.enter_context(tc.tile_pool(name="tt", bufs=2))
        f_pool = ctx.enter_context(tc.tile_pool(name="fin", bufs=2))
        o_pool = ctx.enter_context(tc.tile_pool(name="outs", bufs=2))
        pt_psum = ctx.enter_context(tc.tile_pool(name="ptp", bufs=1, space="PSUM"))
        pg_psum = ctx.enter_context(tc.tile_pool(name="pgp", bufs=2, space="PSUM"))
        tf_psum = ctx.enter_context(tc.tile_pool(name="tfp", bufs=1, space="PSUM"))
        pT_psum = ctx.enter_context(tc.tile_pool(name="pTp", bufs=1, space="PSUM"))
        po_psum = ctx.enter_context(tc.tile_pool(name="pop", bufs=1, space="PSUM"))

        # ---- load constants once ----
        gmats_sb = {}
        for name, d in (("n", gn_d), ("m", gm_d), ("mn", gmn_d)):
            t = cpool.tile([82, GCOLS], dt.bfloat16, tag=f"g{name}")
            nc.gpsimd.dma_start(out=t[:], in_=d[:])
            gmats_sb[name] = t
        wa = cpool.tile([HA, 8 * 82], dt.bfloat16, tag="wa")
        nc.gpsimd.dma_start(out=wa[:], in_=wa_d[:])
        wt = cpool.tile([TROWS, 4 * 82], dt.bfloat16, tag="wt")
        nc.gpsimd.dma_start(out=wt[:], in_=wt_d[:])
        fm = cpool.tile([82, 12], dt.bfloat16, tag="fm")
        nc.gpsimd.dma_start(out=fm[:], in_=fm_d[:])
        ffold = cpool.tile([BP, HB], dt.bfloat16, tag="ffold")
        nc.gpsimd.dma_start(out=ffold[:], in_=ffold_d[:])
        id128 = cpool.tile([128, 128], dt.float32, tag="id128")
        nc.gpsimd.dma_start(out=id128[:], in_=id128_d[:])
        id4 = cpool.tile([4, 4], dt.float32, tag="id4")
        nc.gpsimd.dma_start(out=id4[:], in_=id4_d[:])
        bias_t = cpool.tile([128, 1], dt.float32, tag="biasln")
        nc.vector.memset(bias_t[:], LNHALF)

        def stage12(c):
            b0 = c * NB
            # ---- stage 1: load + transpose -> etT bf16 [82, 512] per comp ----
            nat = {}
            etT = {}
            for comp, src in (("r", xr), ("i", xi)):
                t = nat_pool.tile([128, 4 * 82], dt.float32, tag=f"nat{comp}")
                nc.sync.dma_start(out=t[:].rearrange("p (blk f) -> p blk f", blk=4),
                                  in_=src[b0: b0 + NB, :].rearrange(
                                      "(blk p) f -> p blk f", p=128))
                nat[comp] = t
                ptp = pt_psum.tile([82, NB], dt.float32, tag="tp")
                for blk in range(4):
                    nc.tensor.transpose(ptp[:, blk * 128:(blk + 1) * 128],
                                        t[:, blk * 82:(blk + 1) * 82], id128[:])
                et = et_pool.tile([82, NB], dt.bfloat16, tag=f"et{comp}")
                nc.scalar.copy(et[:], ptp[:])
                etT[comp] = et

            # ---- stage 2: gathers ----
            gt = {}
            for kind in ("n", "m", "mn"):
                for comp in ("r", "i"):
                    for j, (j0, jl) in enumerate(((0, HA), (HA, BP), (HA + BP, HA))):
                        ps = pg_psum.tile([128, NB], dt.float32, tag="gpsum")
                        nc.tensor.matmul(ps[:jl, :], gmats_sb[kind][:, j0:j0 + jl],
                                         etT[comp][:], start=True, stop=True)
                        sb = g_pool.tile([128, NB], dt.bfloat16, tag=f"g{kind}{comp}{j}")
                        a = copy_asg[(kind, comp, j)]
                        if a == "A":
                            nc.scalar.copy(sb[:jl, :], ps[:jl, :])
                        else:
                            nc.vector.tensor_copy(sb[:jl, :], ps[:jl, :])
                        gt[(kind, comp, j)] = sb
            return [b0, nat, etT, gt, None]

        def stage3(state):
            b0, nat, etT, gt = state[:4]
            # ---- stage 3: products + combines -> q1 (S_r), q2 (S_i) per (sname, j) ----
            # S1 pairs En with conj(Emn); S2 pairs Em with conj(Emn).
            q = {}
            for sname, kind in (("S1", "n"), ("S2", "m")):
                for j in range(3):
                    rows = jrows[j]
                    a = pr_pool.tile([128, NB], dt.bfloat16, tag=f"pa{sname}{j}")
                    b_ = pr_pool.tile([128, NB], dt.bfloat16, tag=f"pb{sname}{j}")
                    cc = pr_pool.tile([128, NB], dt.bfloat16, tag=f"pc{sname}{j}")
                    d_ = pr_pool.tile([128, NB], dt.bfloat16, tag=f"pd{sname}{j}")
                    specs = [
                        (a, (kind, "r", j), ("mn", "r", j), 0),
                        (b_, (kind, "i", j), ("mn", "i", j), 1),
                        (cc, (kind, "i", j), ("mn", "r", j), 2),
                        (d_, (kind, "r", j), ("mn", "i", j), 3),
                    ]
                    for out_t, i0, i1, k in specs:
                        eng = nc.gpsimd if (sname, j, k) in POOL_PRODUCTS else nc.vector
                        eng.tensor_mul(out_t[:rows, :], gt[i0][:rows, :], gt[i1][:rows, :])
                    q1 = q_pool.tile([128, NB], dt.bfloat16, tag=f"q1{sname}{j}")
                    q2 = q_pool.tile([128, NB], dt.bfloat16, tag=f"q2{sname}{j}")
                    nc.vector.tensor_add(q1[:rows, :], a[:rows, :], b_[:rows, :])
                    nc.vector.tensor_sub(q2[:rows, :], cc[:rows, :], d_[:rows, :])
                    q[(sname, "q1", j)] = q1
                    q[(sname, "q2", j)] = q2
            state[4] = q

        def stage47(state):
            b0, nat, etT, gt, q = state
            # ---- stage 4: p-folds ----
            qa = {}
            for sname in ("S1", "S2"):
                for qk in ("q1", "q2"):
                    t = q_pool.tile([128, NB], dt.bfloat16, tag=f"qa{sname}{qk}")
                    nc.vector.tensor_add(t[:HA, :], q[(sname, qk, 0)][:HA, :],
                                         q[(sname, qk, 2)][:HA, :])
                    qa[(sname, qk)] = t
            qtl = {}
            for sname in ("S1", "S2"):
                pq = tf_psum.tile([128, NB], dt.float32, tag="tf")
                nc.tensor.matmul(pq[0:HB, :], ffold[:], q[(sname, "q1", 1)][:BP, :],
                                 start=True, stop=True)
                nc.tensor.matmul(pq[64:TROWS, :], ffold[:], q[(sname, "q2", 1)][:BP, :],
                                 start=True, stop=True)
                sb = q_pool.tile([TROWS, NB], dt.bfloat16, tag=f"qt{sname}")
                nc.scalar.copy(sb[:], pq[0:TROWS, :])
                qtl[sname] = sb

            # ---- stage 5: T matmuls -> T_r, T_i psum [82, 512] ----
            tps = {}
            for tgt in ("r", "i"):
                pT = pT_psum.tile([82, NB], dt.float32, tag=f"T{tgt}")
                first = True
                for sname in ("S1", "S2"):
                    for qk in ("q1", "q2"):
                        slot = a_slot[(sname, qk, tgt)]
                        nc.tensor.matmul(pT[:], wa[:, slot * 82:(slot + 1) * 82],
                                         qa[(sname, qk)][:HA, :], start=first, stop=False)
                        first = False
                for sname in ("S1", "S2"):
                    slot = t_slot[(sname, tgt)]
                    last = (sname == "S2")
                    nc.tensor.matmul(pT[:], wt[:, slot * 82:(slot + 1) * 82],
                                     qtl[sname][:], start=False, stop=last)
                tsb = t_pool.tile([82, NB], dt.bfloat16, tag=f"Ts{tgt}")
                nc.scalar.copy(tsb[:], pT[:])
                tps[tgt] = tsb

            # ---- stage 6: final products + ones-reduce -> fo psum [4, 512] ----
            pr1 = f_pool.tile([82, NB], dt.bfloat16, tag="pr1")
            pr2 = f_pool.tile([82, NB], dt.bfloat16, tag="pr2")
            pi1 = f_pool.tile([82, NB], dt.bfloat16, tag="pi1")
            pi2 = f_pool.tile([82, NB], dt.bfloat16, tag="pi2")
            nc.vector.tensor_mul(pr1[:], etT["r"][:], tps["r"][:])
            nc.vector.tensor_mul(pr2[:], etT["i"][:], tps["i"][:])
            nc.vector.tensor_mul(pi1[:], etT["r"][:], tps["i"][:])
            nc.vector.tensor_mul(pi2[:], etT["i"][:], tps["r"][:])
            fo = po_psum.tile([4, NB], dt.float32, tag="fo")
            nc.tensor.matmul(fo[:], fm[:, 0:4], pr1[:], start=True, stop=False)
            nc.tensor.matmul(fo[:], fm[:, 4:8], pr2[:], start=False, stop=False)
            nc.tensor.matmul(fo[:], fm[:, 8:12], pi1[:], start=False, stop=False)
            nc.tensor.matmul(fo[:], fm[:, 8:12], pi2[:], start=False, stop=True)
            fo_sb = f_pool.tile([4, NB], dt.float32, tag="fosb")
            nc.scalar.copy(fo_sb[:], fo[:])

            # ---- stage 7: per-128 block: P-scale + exact E_L add + out ----
            import concourse.mybir as _mb
            tit = o_pool.tile([128, 16], dt.float32, tag="tit")
            nc.sync.dma_start(out=tit[:].rearrange("p (blk c) -> p blk c", blk=4),
                              in_=ti[b0: b0 + NB, :].rearrange(
                                  "(blk p) c -> p blk c", p=128))
            owide = o_pool.tile([128, 16], dt.float32, tag="ow")
            for blk in range(4):
                po = po_psum.tile([128, 4], dt.float32, tag="opsum")
                nc.tensor.transpose(po[:], fo_sb[:, blk * 128:(blk + 1) * 128], id4[:])
                pcol = o_pool.tile([128, 1], dt.float32, tag="pcol")
                nc.scalar.activation(pcol[:], tit[:, 4 * blk: 4 * blk + 1],
                                     _mb.ActivationFunctionType.Exp,
                                     bias=bias_t[:], scale=LN10_10)
                ob = owide[:, 4 * blk: 4 * blk + 4]
                nc.vector.tensor_scalar_mul(ob, po[:], pcol[:])
                # out cols (0,2) += xr cols 2L:2L+2 ; (1,3) += xi cols
                nc.vector.tensor_add(ob[:, 0:4:2], ob[:, 0:4:2],
                                     nat["r"][:, blk * 82 + 2 * L: blk * 82 + 2 * L + 2])
                nc.vector.tensor_add(ob[:, 1:4:2], ob[:, 1:4:2],
                                     nat["i"][:, blk * 82 + 2 * L: blk * 82 + 2 * L + 2])
            nc.sync.dma_start(out=out_d[b0: b0 + NB, :].rearrange(
                "(blk p) c -> p blk c", p=128),
                in_=owide[:].rearrange("p (blk c) -> p blk c", blk=4))

        prev = None
        for c in range(NCHUNK + 1):
            if prev is not None:
                stage3(prev)
            nxt = stage12(c) if c < NCHUNK else None
            if prev is not None:
                stage47(prev)
            prev = nxt

    nc.compile()
    return nc


_CACHE = {}


def _pack_tmats(Wr, Wi):
    mats = _build_tmats(Wr, Wi)
    wa = np.zeros((HA, 8 * 82), dtype=np.float32)
    wt = np.zeros((TROWS, 4 * 82), dtype=np.float32)
    s = 0
    for sname in ("S1", "S2"):
        for qk in ("q1", "q2"):
            for tgt in ("r", "i"):
                wa[:, s * 82:(s + 1) * 82] = mats[("a", sname, qk, tgt)]
                s += 1
    s = 0
    for sname in ("S1", "S2"):
        for tgt in ("r", "i"):
            wt[:, s * 82:(s + 1) * 82] = mats[("t", sname, tgt)]
            s += 1
    return wa.astype(bf16), wt.astype(bf16)


def kernel(xr, xi, task_info, Wr, Wi):
    from concourse.bass_utils import run_bass_kernel_spmd

    xr = np.ascontiguousarray(np.asarray(xr, dtype=np.float32)).reshape(B, 82)
    xi = np.ascontiguousarray(np.asarray(xi, dtype=np.float32)).reshape(B, 82)
    task_info = np.ascontiguousarray(np.asarray(task_info, dtype=np.float32))
    gm = _build_consts()
    Wr32 = np.asarray(Wr, dtype=np.float32)
    Wi32 = np.asarray(Wi, dtype=np.float32)
    wa, wt = _pack_tmats(Wr32, Wi32)
    fr1, fr2, fi = _build_fmats()
    fmp = np.zeros((82, 12), dtype=np.float32)
    fmp[:, 0:4] = fr1
    fmp[:, 4:8] = fr2
    fmp[:, 8:12] = fi
    fmp = fmp.astype(bf16)
    id128 = np.eye(128, dtype=np.float32)
    id4 = np.eye(4, dtype=np.float32)

    if "nc" not in _CACHE:
        _CACHE["nc"] = _build_kernel()
    nc = _CACHE["nc"]

    in_maps = []
    for core in range(NCORES):
        s = slice(core * BC, (core + 1) * BC)
        in_maps.append({
            "xr": xr[s], "xi": xi[s], "ti": task_info[s],
            "gn": gm["n"], "gm": gm["m"], "gmn": gm["mn"],
            "wa": wa, "wt": wt, "fm": fmp, "ffold": _build_ffold2(),
            "id128": id128, "id4": id4,
        })
    res = run_bass_kernel_spmd(nc, in_maps, list(range(NCORES)))
    outs = [res.results[i]["out"] for i in range(NCORES)]
    full = np.concatenate(outs, axis=0)  # [B, 4]
    return full.reshape(B, NMODES, 2).astype(np.float32)


# revision 13
# speedup vs baseline: 1.2481x; 1.2481x over previous
"""Trainium2 Bass kernel for nn_EqPBC (triplet-feature PBC equalizer), v2.

Data-parallel over 8 NeuronCores, 8192 samples/core, 16 chunks of 512.
Per chunk (batch on free dim, features on partitions):
  1. DMA [128,82] f32 blocks, PE-transpose -> etT [82,512] bf16 per comp
  2. One-hot gather matmuls (PE) -> En/Em/Emn rows (p,h); PSUM->SBUF bf16
  3. DVE products + combines: q1=S_r, q2=S_i per (sname in {S1,S2}, j-block)
  4. p-folds: a-blocks on DVE (j0+j2), packed b-tails via PE fold matmul
  5. T-matmuls (PE): T[i,c] = sum_h W'[i,h]*(S1[h]->c=m_h) + (S2[h]->c=n_h)
     with host-built weight matrices; T_r/T_i psum [82,512]
  6. final: Eout = sum_c E[c,i]*T[i,c] -> 4 DVE products + ones-matmuls
  7. out = E_L + Eout * 10^(ti0/10)/2 in f32 (exact E_L), DMA out

OOB Emn indices replicate JAX gather semantics (wrap negatives then clamp).
"""
import numpy as np
import ml_dtypes
from contextlib import ExitStack

# ----- static problem constants (hardcoded; kernel.py must be self-contained) -----
M = 41
L = M // 2
NMODES = 2
B = 65536
NCORES = 8
BC = B // NCORES          # 8192 samples per core
NB = 512                  # samples per chunk
NCHUNK = BC // NB         # 16
THRESH = 1.0 * M // 2
_idx = [(m, n) for m in range(-L, L + 1) for n in range(m, L + 1) if abs(m * n) <= THRESH]
M_ARR = np.array([m for m, n in _idx], dtype=np.int32)
N_ARR = np.array([n for m, n in _idx], dtype=np.int32)
DIAG = np.array([m == n for m, n in _idx])
HDIM = len(_idx)          # 177
HA = 128                  # h-split: a block [0,128), b block [128,177)
HB = HDIM - HA            # 49
BP = 113                  # packed b-block rows: p0 tail at 0:49, p1 tail at 64:113
GCOLS = 128 + BP + 128    # 369
TROWS = 64 + HB           # 113: folded-tail psum rows (q1 at 0:49, q2 at 64:113)

bf16 = ml_dtypes.bfloat16


def _gather_cols(idx_arr):
    """Source row in etT[82,:] (row = 2*(L+c) + p) for gathered rows (p,h)."""
    src = np.empty((2, HDIM), dtype=np.int64)
    for p in range(2):
        src[p] = 2 * (L + idx_arr) + p
    return src


def _build_consts():
    mn = L + M_ARR + N_ARR
    mn = np.clip(np.where(mn < 0, mn + M, mn), 0, M - 1) - L  # jax wrap+clamp
    srcs = {"n": _gather_cols(N_ARR), "m": _gather_cols(M_ARR), "mn": _gather_cols(mn)}
    gmats = {}
    for k, src in srcs.items():
        G = np.zeros((82, GCOLS), dtype=np.float32)
        for p in range(2):
            for h in range(HA):                      # a-blocks
                G[src[p, h], (0 if p == 0 else 128 + BP) + h] = 1.0
            for r in range(HB):                      # packed b-block
                G[src[p, HA + r], 128 + (0 if p == 0 else 64) + r] = 1.0
        gmats[k] = G.astype(bf16)
    return gmats


def _build_ffold2():
    """[113, 98] bf16 used as two [113,49] folds: out[r] = in[r] + in[64+r]."""
    F = np.zeros((BP, HB), dtype=np.float32)
    for r in range(HB):
        F[r, r] = 1.0
        F[64 + r, r] = 1.0
    return F.astype(bf16)


def _build_tmats(Wr, Wi):
    """Weight matrices for the T-stage matmuls.

    T[i,c] = sum_{h: m_h=c} W'[i,h] S1[h] + sum_{h: n_h=c} W'[i,h] S2[h]
    (W' = W * 0.5 on diag).  T row layout = 2*cfreq + i, cfreq = L + c.
    Returns dict of lhsT mats:
      ('a', sname, qk, tgt): [128, 82]  for p-folded a-block q tiles
      ('t', sname, tgt):     [98, 82]   for folded-tail tiles (q1 0:49, q2 49:98)
    tgt in {'r','i'}; q1 rows are S_r, q2 rows are S_i:
      T_r <- +W'_r*q1 - W'_i*q2 ; T_i <- +W'_i*q1 + W'_r*q2
    """
    scale = np.where(DIAG, 0.5, 1.0).astype(np.float32)
    wr = Wr * scale[None, :]  # [2, HDIM]
    wi = Wi * scale[None, :]
    cidx = {"S1": L + M_ARR, "S2": L + N_ARR}  # output freq index per h
    mats = {}
    for sname in ("S1", "S2"):
        c = cidx[sname]
        for tgt in ("r", "i"):
            # a-block: rows h in [0,128)
            m1 = np.zeros((HA, 82), dtype=np.float32)   # for q1 (S_r)
            m2 = np.zeros((HA, 82), dtype=np.float32)   # for q2 (S_i)
            for h in range(HA):
                for i in range(2):
                    col = 2 * c[h] + i
                    if tgt == "r":
                        m1[h, col] += wr[i, h]
                        m2[h, col] += -wi[i, h]
                    else:
                        m1[h, col] += wi[i, h]
                        m2[h, col] += wr[i, h]
            mats[("a", sname, "q1", tgt)] = m1.astype(bf16)
            mats[("a", sname, "q2", tgt)] = m2.astype(bf16)
            # tail: rows 0:49 = q1 (h=128+r), rows 64:113 = q2
            mt = np.zeros((TROWS, 82), dtype=np.float32)
            for r in range(HB):
                h = HA + r
                for i in range(2):
                    col = 2 * c[h] + i
                    if tgt == "r":
                        mt[r, col] += wr[i, h]
                        mt[64 + r, col] += -wi[i, h]
                    else:
                        mt[r, col] += wi[i, h]
                        mt[64 + r, col] += wr[i, h]
            mats[("t", sname, tgt)] = mt.astype(bf16)
    return mats


def _build_fmats():
    """Final reduce mats [82,4] bf16. out rows: 0=m0r,1=m0i,2=m1r,3=m1i.
    out_r[i] = sum_c (PR1 - PR2)[2c+i]; out_i[i] = sum_c (PI1 + PI2)[2c+i]."""
    fr1 = np.zeros((82, 4), dtype=np.float32)
    fr2 = np.zeros((82, 4), dtype=np.float32)
    fi = np.zeros((82, 4), dtype=np.float32)
    for c in range(M):
        for i in range(2):
            fr1[2 * c + i, 2 * i] = 1.0
            fr2[2 * c + i, 2 * i] = -1.0
            fi[2 * c + i, 2 * i + 1] = 1.0
    return fr1.astype(bf16), fr2.astype(bf16), fi.astype(bf16)


# engine assignment for gather-tile PSUM->SBUF copies: (kind, comp, j) -> "A"|"P"
def _copy_assignment():
    # Pool cannot read PSUM; split copies between Act ("A") and DVE ("D")
    asg = {}
    dve_set = {("n", "r", 1), ("n", "i", 1), ("m", "r", 1), ("m", "i", 1)}
    for kind in ("n", "m", "mn"):
        for comp in ("r", "i"):
            for j in range(3):
                asg[(kind, comp, j)] = "D" if (kind, comp, j) in dve_set else "A"
    return asg


# products that run on Pool instead of DVE: set of (sname, j, k) with k in 0..3
POOL_PRODUCTS = {("S2", j, k) for j in range(3) for k in range(4)}


def _build_kernel():
    import concourse.bass as bass
    import concourse.bacc as bacc
    import concourse.tile as tile
    import concourse.mybir as mybir

    dt = mybir.dt
    nc = bacc.Bacc("TRN2", target_bir_lowering=False, debug=False, num_devices=NCORES)
    xr = nc.declare_dram_parameter("xr", [BC, 82], dt.float32, isOutput=False)
    xi = nc.declare_dram_parameter("xi", [BC, 82], dt.float32, isOutput=False)
    ti = nc.declare_dram_parameter("ti", [BC, 4], dt.float32, isOutput=False)
    gn_d = nc.declare_dram_parameter("gn", [82, GCOLS], dt.bfloat16, isOutput=False)
    gm_d = nc.declare_dram_parameter("gm", [82, GCOLS], dt.bfloat16, isOutput=False)
    gmn_d = nc.declare_dram_parameter("gmn", [82, GCOLS], dt.bfloat16, isOutput=False)
    # T-stage weight mats packed: a-mats [128, 8*82], tail-mats [98, 4*82]
    wa_d = nc.declare_dram_parameter("wa", [HA, 8 * 82], dt.bfloat16, isOutput=False)
    wt_d = nc.declare_dram_parameter("wt", [TROWS, 4 * 82], dt.bfloat16, isOutput=False)
    fm_d = nc.declare_dram_parameter("fm", [82, 12], dt.bfloat16, isOutput=False)
    ffold_d = nc.declare_dram_parameter("ffold", [BP, HB], dt.bfloat16, isOutput=False)
    id128_d = nc.declare_dram_parameter("id128", [128, 128], dt.float32, isOutput=False)
    id4_d = nc.declare_dram_parameter("id4", [4, 4], dt.float32, isOutput=False)
    out_d = nc.declare_dram_parameter("out", [BC, 4], dt.float32, isOutput=True)

    LN10_10 = float(np.log(10.0) / 10.0)
    LNHALF = float(np.log(0.5))

    # a-mat column layout: idx = (sname,qk,tgt) -> slot
    a_slot = {}
    t_slot = {}
    s = 0
    for sname in ("S1", "S2"):
        for qk in ("q1", "q2"):
            for tgt in ("r", "i"):
                a_slot[(sname, qk, tgt)] = s
                s += 1
    s = 0
    for sname in ("S1", "S2"):
        for tgt in ("r", "i"):
            t_slot[(sname, tgt)] = s
            s += 1

    copy_asg = _copy_assignment()
    jrows = [HA, BP, HA]

    with tile.TileContext(nc) as tc, ExitStack() as ctx:
        cpool = ctx.enter_context(tc.tile_pool(name="consts", bufs=1))
        nat_pool = ctx.enter_context(tc.tile_pool(name="nat", bufs=3))
        et_pool = ctx.enter_context(tc.tile_pool(name="et", bufs=3))
        g_pool = ctx.enter_context(tc.tile_pool(name="gath", bufs=2))
        pr_pool = ctx.enter_context(tc.tile_pool(name="prod", bufs=2))
        q_pool = ctx.enter_context(tc.tile_pool(name="qt", bufs=3))
        t_pool = ctx.enter_context(tc.tile_pool(name="tt", bufs=2))
        f_pool = ctx.enter_context(tc.tile_pool(name="fin", bufs=2))
        o_pool = ctx.enter_context(tc.tile_pool(name="outs", bufs=2))
        pt_psum = ctx.enter_context(tc.tile_pool(name="ptp", bufs=1, space="PSUM"))
        pg_psum = ctx.enter_context(tc.tile_pool(name="pgp", bufs=2, space="PSUM"))
        tf_psum = ctx.enter_context(tc.tile_pool(name="tfp", bufs=1, space="PSUM"))
        pT_psum = ctx.enter_context(tc.tile_pool(name="pTp", bufs=1, space="PSUM"))
        po_psum = ctx.enter_context(tc.tile_pool(name="pop", bufs=1, space="PSUM"))

        # ---- load constants once ----
        gmats_sb = {}
        for name, d in (("n", gn_d), ("m", gm_d), ("mn", gmn_d)):
            t = cpool.tile([82, GCOLS], dt.bfloat16, tag=f"g{name}")
            nc.gpsimd.dma_start(out=t[:], in_=d[:])
            gmats_sb[name] = t
        wa = cpool.tile([HA, 8 * 82], dt.bfloat16, tag="wa")
        nc.gpsimd.dma_start(out=wa[:], in_=wa_d[:])
        wt = cpool.tile([TROWS, 4 * 82], dt.bfloat16, tag="wt")
        nc.gpsimd.dma_start(out=wt[:], in_=wt_d[:])
        fm = cpool.tile([82, 12], dt.bfloat16, tag="fm")
        nc.gpsimd.dma_start(out=fm[:], in_=fm_d[:])
        ffold = cpool.tile([BP, HB], dt.bfloat16, tag="ffold")
        nc.gpsimd.dma_start(out=ffold[:], in_=ffold_d[:])
        id128 = cpool.tile([128, 128], dt.float32, tag="id128")
        nc.gpsimd.dma_start(out=id128[:], in_=id128_d[:])
        id4 = cpool.tile([4, 4], dt.float32, tag="id4")
        nc.gpsimd.dma_start(out=id4[:], in_=id4_d[:])
        bias_t = cpool.tile([128, 1], dt.float32, tag="biasln")
        nc.vector.memset(bias_t[:], LNHALF)

        def stage12(c):
            b0 = c * NB
            dve_copies = []
            # ---- stage 1: load + transpose -> etT bf16 [82, 512] per comp ----
            nat = {}
            etT = {}
            for comp, src in (("r", xr), ("i", xi)):
                t = nat_pool.tile([128, 4 * 82], dt.float32, tag=f"nat{comp}")
                nc.sync.dma_start(out=t[:].rearrange("p (blk f) -> p blk f", blk=4),
                                  in_=src[b0: b0 + NB, :].rearrange(
                                      "(blk p) f -> p blk f", p=128))
                nat[comp] = t
                ptp = pt_psum.tile([82, NB], dt.float32, tag="tp")
                for blk in range(4):
                    nc.tensor.transpose(ptp[:, blk * 128:(blk + 1) * 128],
                                        t[:, blk * 82:(blk + 1) * 82], id128[:])
                et = et_pool.tile([82, NB], dt.bfloat16, tag=f"et{comp}")
                nc.scalar.copy(et[:], ptp[:])
                etT[comp] = et

            # ---- stage 2+3: gathers fused with products per j-block ----
            gt = {}
            q = {}
            jsl = ((0, HA), (HA, BP), (HA + BP, HA))
            for j in range(3):
                j0, jl = jsl[j]
                for kind in ("n", "m", "mn"):
                    for comp in ("r", "i"):
                        ps = pg_psum.tile([128, NB], dt.float32, tag="gpsum")
                        nc.tensor.matmul(ps[:jl, :], gmats_sb[kind][:, j0:j0 + jl],
                                         etT[comp][:], start=True, stop=True)
                        sb = g_pool.tile([128, NB], dt.bfloat16, tag=f"g{kind}{comp}{j}")
                        a = copy_asg[(kind, comp, j)]
                        if a == "A":
                            nc.scalar.copy(sb[:jl, :], ps[:jl, :])
                        else:
                            nc.vector.tensor_copy(sb[:jl, :], ps[:jl, :])
                        gt[(kind, comp, j)] = sb
                rows = jl
                for sname, kind in (("S1", "n"), ("S2", "m")):
                    a_ = pr_pool.tile([128, NB], dt.bfloat16, tag=f"pa{sname}{j}")
                    b_ = pr_pool.tile([128, NB], dt.bfloat16, tag=f"pb{sname}{j}")
                    cc = pr_pool.tile([128, NB], dt.bfloat16, tag=f"pc{sname}{j}")
                    d_ = pr_pool.tile([128, NB], dt.bfloat16, tag=f"pd{sname}{j}")
                    specs = [
                        (a_, (kind, "r", j), ("mn", "r", j), 0),
                        (b_, (kind, "i", j), ("mn", "i", j), 1),
                        (cc, (kind, "i", j), ("mn", "r", j), 2),
                        (d_, (kind, "r", j), ("mn", "i", j), 3),
                    ]
                    for out_t, i0, i1, k in specs:
                        eng = nc.gpsimd if (sname, j, k) in POOL_PRODUCTS else nc.vector
                        eng.tensor_mul(out_t[:rows, :], gt[i0][:rows, :], gt[i1][:rows, :])
                    q1 = q_pool.tile([128, NB], dt.bfloat16, tag=f"q1{sname}{j}")
                    q2 = q_pool.tile([128, NB], dt.bfloat16, tag=f"q2{sname}{j}")
                    nc.vector.tensor_add(q1[:rows, :], a_[:rows, :], b_[:rows, :])
                    nc.vector.tensor_sub(q2[:rows, :], cc[:rows, :], d_[:rows, :])
                    q[(sname, "q1", j)] = q1
                    q[(sname, "q2", j)] = q2
            return [b0, nat, etT, gt, q]

        def stage47(state):
            b0, nat, etT, gt, q = state
            # ---- stage 4: p-folds ----
            qa = {}
            for sname in ("S1", "S2"):
                for qk in ("q1", "q2"):
                    t = q_pool.tile([128, NB], dt.bfloat16, tag=f"qa{sname}{qk}")
                    nc.vector.tensor_add(t[:HA, :], q[(sname, qk, 0)][:HA, :],
                                         q[(sname, qk, 2)][:HA, :])
                    qa[(sname, qk)] = t
            qtl = {}
            for sname in ("S1", "S2"):
                pq = tf_psum.tile([128, NB], dt.float32, tag="tf")
                nc.tensor.matmul(pq[0:HB, :], ffold[:], q[(sname, "q1", 1)][:BP, :],
                                 start=True, stop=True)
                nc.tensor.matmul(pq[64:TROWS, :], ffold[:], q[(sname, "q2", 1)][:BP, :],
                                 start=True, stop=True)
                sb = q_pool.tile([TROWS, NB], dt.bfloat16, tag=f"qt{sname}")
                nc.scalar.copy(sb[:], pq[0:TROWS, :])
                qtl[sname] = sb

            # ---- stage 5: T matmuls -> T_r, T_i psum [82, 512] ----
            tps = {}
            for tgt in ("r", "i"):
                pT = pT_psum.tile([82, NB], dt.float32, tag=f"T{tgt}")
                first = True
                for sname in ("S1", "S2"):
                    for qk in ("q1", "q2"):
                        slot = a_slot[(sname, qk, tgt)]
                        nc.tensor.matmul(pT[:], wa[:, slot * 82:(slot + 1) * 82],
                                         qa[(sname, qk)][:HA, :], start=first, stop=False)
                        first = False
                for sname in ("S1", "S2"):
                    slot = t_slot[(sname, tgt)]
                    last = (sname == "S2")
                    nc.tensor.matmul(pT[:], wt[:, slot * 82:(slot + 1) * 82],
                                     qtl[sname][:], start=False, stop=last)
                tsb = t_pool.tile([82, NB], dt.bfloat16, tag=f"Ts{tgt}")
                nc.scalar.copy(tsb[:], pT[:])
                tps[tgt] = tsb

            # ---- stage 6: final products + ones-reduce -> fo psum [4, 512] ----
            pr1 = f_pool.tile([82, NB], dt.bfloat16, tag="pr1")
            pr2 = f_pool.tile([82, NB], dt.bfloat16, tag="pr2")
            pi1 = f_pool.tile([82, NB], dt.bfloat16, tag="pi1")
            pi2 = f_pool.tile([82, NB], dt.bfloat16, tag="pi2")
            nc.vector.tensor_mul(pr1[:], etT["r"][:], tps["r"][:])
            nc.vector.tensor_mul(pr2[:], etT["i"][:], tps["i"][:])
            nc.vector.tensor_mul(pi1[:], etT["r"][:], tps["i"][:])
            nc.vector.tensor_mul(pi2[:], etT["i"][:], tps["r"][:])
            fo = po_psum.tile([4, NB], dt.float32, tag="fo")
            nc.tensor.matmul(fo[:], fm[:, 0:4], pr1[:], start=True, stop=False)
            nc.tensor.matmul(fo[:], fm[:, 4:8], pr2[:], start=False, stop=False)
            nc.tensor.matmul(fo[:], fm[:, 8:12], pi1[:], start=False, stop=False)
            nc.tensor.matmul(fo[:], fm[:, 8:12], pi2[:], start=False, stop=True)
            fo_sb = f_pool.tile([4, NB], dt.float32, tag="fosb")
            nc.scalar.copy(fo_sb[:], fo[:])

            # ---- stage 7: per-128 block: P-scale + exact E_L add + out ----
            import concourse.mybir as _mb
            tit = o_pool.tile([128, 16], dt.float32, tag="tit")
            nc.sync.dma_start(out=tit[:].rearrange("p (blk c) -> p blk c", blk=4),
                              in_=ti[b0: b0 + NB, :].rearrange(
                                  "(blk p) c -> p blk c", p=128))
            owide = o_pool.tile([128, 16], dt.float32, tag="ow")
            for blk in range(4):
                po = po_psum.tile([128, 4], dt.float32, tag="opsum")
                nc.tensor.transpose(po[:], fo_sb[:, blk * 128:(blk + 1) * 128], id4[:])
                pcol = o_pool.tile([128, 1], dt.float32, tag="pcol")
                nc.scalar.activation(pcol[:], tit[:, 4 * blk: 4 * blk + 1],
                                     _mb.ActivationFunctionType.Exp,
                                     bias=bias_t[:], scale=LN10_10)
                ob = owide[:, 4 * blk: 4 * blk + 4]
                nc.vector.tensor_scalar_mul(ob, po[:], pcol[:])
                # out cols (0,2) += xr cols 2L:2L+2 ; (1,3) += xi cols
                nc.vector.tensor_add(ob[:, 0:4:2], ob[:, 0:4:2],
                                     nat["r"][:, blk * 82 + 2 * L: blk * 82 + 2 * L + 2])
                nc.vector.tensor_add(ob[:, 1:4:2], ob[:, 1:4:2],
                                     nat["i"][:, blk * 82 + 2 * L: blk * 82 + 2 * L + 2])
            nc.sync.dma_start(out=out_d[b0: b0 + NB, :].rearrange(
                "(blk p) c -> p blk c", p=128),
                in_=owide[:].rearrange("p (blk c) -> p blk c", blk=4))

        prev = None
        for c in range(NCHUNK + 1):
            nxt = stage12(c) if c < NCHUNK else None
            if prev is not None:
                stage47(prev)
            prev = nxt

    nc.compile()
    return nc


_CACHE = {}


def _pack_tmats(Wr, Wi):
    mats = _build_tmats(Wr, Wi)
    wa = np.zeros((HA, 8 * 82), dtype=np.float32)
    wt = np.zeros((TROWS, 4 * 82), dtype=np.float32)
    s = 0
    for sname in ("S1", "S2"):
        for qk in ("q1", "q2"):
            for tgt in ("r", "i"):
                wa[:, s * 82:(s + 1) * 82] = mats[("a", sname, qk, tgt)]
                s += 1
    s = 0
    for sname in ("S1", "S2"):
        for tgt in ("r", "i"):
            wt[:, s * 82:(s + 1) * 82] = mats[("t", sname, tgt)]
            s += 1
    return wa.astype(bf16), wt.astype(bf16)


def kernel(xr, xi, task_info, Wr, Wi):
    from concourse.bass_utils import run_bass_kernel_spmd

    xr = np.ascontiguousarray(np.asarray(xr, dtype=np.float32)).reshape(B, 82)
    xi = np.ascontiguousarray(np.asarray(xi, dtype=np.float32)).reshape(B, 82)
    task_info = np.ascontiguousarray(np.asarray(task_info, dtype=np.float32))
    gm = _build_consts()
    Wr32 = np.asarray(Wr, dtype=np.float32)
    Wi32 = np.asarray(Wi, dtype=np.float32)
    wa, wt = _pack_tmats(Wr32, Wi32)
    fr1, fr2, fi = _build_fmats()
    fmp = np.zeros((82, 12), dtype=np.float32)
    fmp[:, 0:4] = fr1
    fmp[:, 4:8] = fr2
    fmp[:, 8:12] = fi
    fmp = fmp.astype(bf16)
    id128 = np.eye(128, dtype=np.float32)
    id4 = np.eye(4, dtype=np.float32)

    if "nc" not in _CACHE:
        _CACHE["nc"] = _build_kernel()
    nc = _CACHE["nc"]

    in_maps = []
    for core in range(NCORES):
        s = slice(core * BC, (core + 1) * BC)
        in_maps.append({
            "xr": xr[s], "xi": xi[s], "ti": task_info[s],
            "gn": gm["n"], "gm": gm["m"], "gmn": gm["mn"],
            "wa": wa, "wt": wt, "fm": fmp, "ffold": _build_ffold2(),
            "id128": id128, "id4": id4,
        })
    res = run_bass_kernel_spmd(nc, in_maps, list(range(NCORES)))
    outs = [res.results[i]["out"] for i in range(NCORES)]
    full = np.concatenate(outs, axis=0)  # [B, 4]
    return full.reshape(B, NMODES, 2).astype(np.float32)
